# revision 50
# baseline (speedup 1.0000x reference)
"""Causal self-attention (B=4, T=2048, C=768, H=12) on 8 trn2 NeuronCores.

Sharding: core c -> batch b = c//2, head-half hh = c%2 (6 heads per core).
Each core computes, for its (b, 6 heads): qkv projection, causal attention,
and a partial output projection (its heads' rows of W_proj). The host sums
the two partial outputs per batch and adds b_proj.

v11 (190us) over the 222us v5 baseline:
  - ALL inputs re-laid-out on the host so every input DMA is a fully
    contiguous 2D block on both the DRAM and SBUF side (column-sliced
    patterns ran at ~35-75 GB/s because of 256-768B lines; contiguous
    runs hit full rate). The two hwdge queues (sync + scalar) share
    ~358 GB/s and the scalar queue has a ~5us slow start, so each queue
    is ordered by arrival need with the phase-1 prefix (pair-0 w_qk on
    scalar, x chunk 0 on sync) first.
  - phase 1 = attn0-chunk0 prerequisites only (q/k cols 0:512, v tiles
    0-3); the rest of qkv is chunk fillers. Attention starts ~18us and
    the HAM clock stays at 8/8 for the whole kernel body.
  - exp alternates FULL-WIDTH ops between Vector (Schraudolph
    bits-of-fp16) and Scalar ACT per block: full-width amortizes the
    ~340ns per-op engine overhead, and alternation gives each engine two
    block-times per exp. The PV software pipeline is depth 2 (PV(i)
    emitted after S(i+2)) so exp latency never gates the PE (v5-v7
    stalled 0.6-2.3us per block behind exp in diag-heavy regions ->
    14-17us HAM cold windows).
  - vector-exp diag blocks need NO PE mask matmul: the causal mask is
    fused into the Schraudolph op (scalar_tensor_tensor with a bias tile
    whose masked entries hold SCH_B - 45*SCH_A; the int16 result
    saturates to -32768 -> bitcast fp16 -0.0, contributing -0 to PV).
    The [lo:1024] exp window puts both diagonal sub-blocks at fixed view
    offsets 0 and 512, so one 1024-col bias tile serves every lo.
    Scalar-exp diag blocks keep the -45 mask matmul (ACT exp -> 0).
  - normalize partition-swap via a tiny PE permutation matmul on fp16
    staged denominators (v5 used SBUF->SBUF DMAs, which queue behind
    proj output DMAs on the sync queue at chunk boundaries), then one
    [128,512] reciprocal covers both heads.
  - copies split across engines: s1 staging A/scalar + B/vector, proj
    copy group1/scalar + group2/vector, qk-unit copies alternate.
  - output written fp16, DMA'd per 512/256-col group (halves out DMA and
    overlaps the final tile's copy; host sums partials in f32).
"""

import numpy as np

B, T, C = 4, 2048, 768
H = 12
D = C // H          # 64
HPC = 6             # heads per core
NP = 3              # head pairs per core
N_CORES = 8
TK = T // 128       # 16 tk tiles
NCH = T // 512      # 4 tq chunks
CT = C // 128       # 6 contraction tiles

SCH_A = 1477.319722   # 1024/ln2
SCH_B = 15301.086468  # 15*1024 - mean-centering constant

_cache = {}


def _build(has_bias):
    import concourse.tile as tile
    from concourse import bacc, mybir

    dt = mybir.dt
    f32 = dt.float32
    f16 = dt.float16
    i16 = dt.int16
    Exp = mybir.ActivationFunctionType.Exp
    Alu = mybir.AluOpType

    nc = bacc.Bacc("TRN2", target_bir_lowering=False, debug=False,
                   num_devices=N_CORES)

    # host-re-laid-out inputs (see _prep_inputs): all DMAs contiguous.
    xe_ap = nc.dram_tensor("xe", [512, CT * 512], f16, kind="ExternalInput").ap()
    we_ap = nc.dram_tensor("we", [128, 6912], f16, kind="ExternalInput").ap()
    wpe_ap = nc.dram_tensor("wpe", [128, NP * C], f16, kind="ExternalInput").ap()
    cm_ap = nc.dram_tensor("cm", [128, 1536], f16, kind="ExternalInput").ap()
    if has_bias:
        ox_ap = nc.dram_tensor("ox", [1, T], f16, kind="ExternalInput").ap()
        wb_ap = nc.dram_tensor("wb", [1, 1152], f16, kind="ExternalInput").ap()
    out_ap = nc.dram_tensor("out", [T, C], f16, kind="ExternalOutput").ap()

    with tile.TileContext(nc) as tc:
        with tc.tile_pool(name="pers", bufs=1) as pers, \
             tc.tile_pool(name="pP", bufs=6) as pP, \
             tc.tile_pool(name="pst", bufs=2) as pst, \
             tc.tile_pool(name="pout", bufs=2) as pout, \
             tc.tile_pool(name="psA", bufs=2, space="PSUM") as psA, \
             tc.tile_pool(name="psB", bufs=2, space="PSUM") as psB, \
             tc.tile_pool(name="psV", bufs=2, space="PSUM") as psV:

            # ---- persistent SBUF tensors ----
            scr = pers.tile([128, 256], f16, tag="scr")
            nc.gpsimd.memset(scr[:], 0.001)
            cmt = pers.tile([128, 1536], f16, tag="cmt")
            xb = pers.tile([128, 4 * 3072], f16, tag="xb", name="xb")
            wab = pers.tile([128, 6912], f16, tag="wab", name="wab")
            wpb = pers.tile([128, NP * C], f16, tag="wpb", name="wpb")
            nid = cmt[:, 0:128]       # -45 * I_128
            mk2 = cmt[:, 128:384]     # [tril(-1) | tril(-1)]
            perm = cmt[:, 384:512]    # half-swap permutation [[0,I],[I,0]]
            # Schraudolph bias-plus-mask tile: SCH_B everywhere except the
            # two (lo-invariant) diagonal sub-block windows, where masked
            # entries carry SCH_B - 45*SCH_A (saturates int16 -> -0.0).
            smk = cmt[:, 512:1536]

            # SBUF layout readers
            def xq(c, cp2):           # qk-unit rhs: x cols cp2*512, tile c
                o = cp2 * 3072 + c * 512
                return xb[:, o:o + 512]

            def xv(c, t):             # v-unit lhsT: x cols t*128, tile c
                o = (t // 4) * 3072 + c * 512 + (t % 4) * 128
                return xb[:, o:o + 128]

            def waq(p, qsel, c):      # qk-unit stationary column block
                if p == 0:
                    o = (0 if qsel == 0 else 768) + c * 128
                else:
                    o = (3840 if qsel == 0 else 5376) + c * 256 + (p - 1) * 128
                return wab[:, o:o + 128]

            def wav(c):               # v-unit rhs columns
                o = 1536 + c * 384
                return wab[:, o:o + 384]

            # ---- input DMA: two hwdge queues share ~358 GB/s; the sync
            # queue is measurably faster and the scalar queue has a ~5us
            # slow start, so the phase-1 gate (pair-0 w_qk) leads the
            # scalar queue while x ch0 leads sync.
            nc.scalar.dma_start(wab[:, 0:1536], we_ap[:, 0:1536])       # p0 qk
            nc.scalar.dma_start(cmt[:], cm_ap)
            nc.scalar.dma_start(xb[:, 2 * 3072:3 * 3072],
                                xe_ap[2 * 128:3 * 128, :])               # x ch2
            nc.scalar.dma_start(wab[:, 3840:6912], we_ap[:, 3840:6912])  # p12
            nc.scalar.dma_start(wpb[:], wpe_ap)
            if has_bias:
                ox = pers.tile([1, T], f16, tag="ox")
                nc.scalar.dma_start(ox[:], ox_ap)
                wb = pers.tile([1, 1152], f16, tag="wb")
                nc.scalar.dma_start(wb[:], wb_ap)
            nc.sync.dma_start(xb[:, 0:3072], xe_ap[0:128, :])            # x ch0
            nc.sync.dma_start(wab[:, 1536:3840], we_ap[:, 1536:3840])    # v cols
            nc.sync.dma_start(xb[:, 3072:2 * 3072], xe_ap[128:256, :])   # x ch1
            nc.sync.dma_start(xb[:, 3 * 3072:4 * 3072],
                              xe_ap[3 * 128:4 * 128, :])                 # x ch3

            qk = [pers.tile([128, T], f16, tag=f"qk{i}", name=f"qk{i}")
                  for i in range(2 * NP)]
            # PV stationaries per t-tile: [128, 768], head h at cols
            # h*128..h*128+128 as [v | ones] (even h) / [ones | v] (odd h).
            voT = [pers.tile([128, HPC * 128], f16, tag=f"v{t}", name=f"v{t}")
                   for t in range(TK)]
            for t in range(TK):
                d4 = voT[t][:].rearrange("p (a h x) -> p a h x", a=3, h=2)
                nc.gpsimd.memset(d4[:, :, 0, 64:128], 1.0)
                nc.gpsimd.memset(d4[:, :, 1, 0:64], 1.0)

            def vo(t, h):
                return voT[t][:, h * 128:(h + 1) * 128]

            # Y^T for pair p: fresh tile for p=0; pairs 1 and 2 reuse the
            # q^T tiles of pairs 0 and 1, which are dead by the time attn
            # p starts writing (Tile tracks the WAR dependency).
            y0 = pers.tile([128, T], f16, tag="y0", name="y0")
            yt = [y0, qk[0], qk[2]]

            # ---- qkv projection / proj emit units ----
            cp_ctr = [0]

            def emit_qk_unit(p, u, on_act=False):
                qsel, cp2 = u // 4, u % 4    # qsel: 0=q 1=k; cp2: 512-col blk
                dst = qk[2 * p + qsel]
                t0 = cp2 * 512
                ps = psB.tile([128, 512], f32, tag="B")
                for c in range(CT):
                    nc.tensor.matmul(
                        ps[:], lhsT=waq(p, qsel, c), rhs=xq(c, cp2),
                        start=(c == 0),
                        stop=(c == CT - 1 and not has_bias))
                if has_bias:
                    wcol = qsel * 384 + p * 128
                    nc.tensor.matmul(
                        ps[:], lhsT=wb[0:1, wcol:wcol + 128],
                        rhs=ox[0:1, t0:t0 + 512],
                        start=False, stop=True)
                cp_ctr[0] += 1
                if on_act or cp_ctr[0] % 2 == 0:
                    nc.scalar.copy(dst[:, t0:t0 + 512], ps[:])
                else:
                    nc.vector.tensor_copy(dst[:, t0:t0 + 512], ps[:])

            def emit_v_unit(t):
                ps = psB.tile([128, 512], f32, tag="B")
                for c in range(CT):
                    nc.tensor.matmul(
                        ps[:, 0:384], lhsT=xv(c, t), rhs=wav(c),
                        start=(c == 0),
                        stop=(c == CT - 1 and not has_bias))
                if has_bias:
                    nc.tensor.matmul(
                        ps[:, 0:384],
                        lhsT=ox[0:1, t * 128:(t + 1) * 128],
                        rhs=wb[0:1, 768:1152],
                        start=False, stop=True)
                s4 = ps[:, 0:384].rearrange("p (a h x) -> p a h x", a=3, h=2)
                d4 = voT[t][:].rearrange("p (a h x) -> p a h x", a=3, h=2)
                nc.vector.tensor_copy(d4[:, :, 0, 0:64], s4[:, :, 0, :])
                nc.vector.tensor_copy(d4[:, :, 1, 64:128], s4[:, :, 1, :])

            def emit_proj_t(t, big=False):
                # out DMA split per column group so the first half's
                # transfer overlaps the second half's copy. big=True (the
                # final chunk, when psA is no longer cycling S tiles) puts
                # both groups in one psA tile so the copies don't
                # serialize the next tile's matmuls through the psB WAR.
                ob = pout.tile([128, C], f16, tag="o")
                pa = None
                if big:
                    pa = psA.tile([128, 1024], f32, tag="A", name="pa")
                for gi, (n0, n1) in enumerate(((0, 512), (512, 768))):
                    if big:
                        pp = pa[:, n0:n1]
                    else:
                        pb = psB.tile([128, 512], f32, tag="B", name="pb")
                        pp = pb[:, 0:n1 - n0]
                    for kk in range(NP):
                        nc.tensor.matmul(
                            pp, lhsT=yt[kk][:, t * 128:(t + 1) * 128],
                            rhs=wpb[:, kk * C + n0:kk * C + n1],
                            start=(kk == 0), stop=(kk == NP - 1))
                    if gi == 0:
                        nc.scalar.copy(ob[:, n0:n1], pp)
                    else:
                        nc.vector.tensor_copy(ob[:, n0:n1], pp)
                    nc.sync.dma_start(
                        out_ap[t * 128:(t + 1) * 128, n0:n1], ob[:, n0:n1])

            def dummy_mms(n):
                # PE keep-warm filler: no data deps, writes a scratch slot.
                wd = psV.tile([128, 512], f32, tag="pv")
                for w in range(n):
                    nc.tensor.matmul(wd[:, 0:256], lhsT=scr[:, 0:128],
                                     rhs=scr[:], start=True, stop=True)

            # ---- phase 1: HAM warmup + attn0-chunk0 prerequisites ----
            with nc.named_scope("ph1"):
                dummy_mms(24)
                emit_qk_unit(0, 0, on_act=True)
                emit_qk_unit(0, 4, on_act=True)
                for t in range(4):
                    emit_v_unit(t)

            def qk_filler(p, u):
                return lambda: emit_qk_unit(p, u)

            def v_filler(t):
                return lambda: emit_v_unit(t)

            def pj_filler(t):
                return lambda: emit_proj_t(t)

            FILL = {
                (0, 0): [qk_filler(0, 1), qk_filler(0, 5),
                         v_filler(4), v_filler(5), v_filler(6), v_filler(7)],
                (0, 1): [qk_filler(0, 2), qk_filler(0, 6),
                         v_filler(8), v_filler(9), v_filler(10), v_filler(11)],
                (0, 2): [qk_filler(0, 3), qk_filler(0, 7),
                         v_filler(12), v_filler(13), v_filler(14), v_filler(15)],
                (0, 3): [qk_filler(1, 0), qk_filler(1, 4)],
                (1, 0): [qk_filler(1, 1), qk_filler(1, 5)],
                (1, 1): [qk_filler(1, 2), qk_filler(1, 6)],
                (1, 2): [qk_filler(1, 3), qk_filler(1, 7)],
                (1, 3): [qk_filler(2, 0), qk_filler(2, 4)],
                (2, 0): [qk_filler(2, 1), qk_filler(2, 5)],
                (2, 1): [qk_filler(2, 2), qk_filler(2, 6),
                         pj_filler(0), pj_filler(1), pj_filler(2), pj_filler(3)],
                (2, 2): [qk_filler(2, 3), qk_filler(2, 7),
                         pj_filler(4), pj_filler(5)],
                (2, 3): [pj_filler(6), pj_filler(7), pj_filler(8),
                         pj_filler(9), pj_filler(10), pj_filler(11)],
            }

            # ---- phase 2: attention ----
            exp_ctr = [0]
            for p in range(NP):
                qA = qk[2 * p]
                kA = qk[2 * p + 1]
                with nc.named_scope(f"attn{p}"):
                    for j in range(NCH):
                        nblk = 4 * j + 4
                        fillers = FILL[(p, j)]
                        nf = len(fillers)
                        emitted = [0]
                        hold = min(2, nf)   # keep some for the chunk end

                        def pump(i):
                            want = min(nf - hold,
                                       (i + 1) * (nf - hold) // max(nblk - 1, 1))
                            while emitted[0] < want:
                                fillers[emitted[0]]()
                                emitted[0] += 1

                        pvA = psV.tile([128, 512], f32, tag="pv")
                        pvB = psV.tile([128, 512], f32, tag="pv")
                        Ps = [None] * nblk
                        ms = [None] * nblk

                        def emit_S(i):
                            m = i - 4 * j
                            lo = 128 * m if m >= 0 else 0
                            exp_ctr[0] += 1
                            on_vec = exp_ctr[0] % 2 == 1
                            mask_mm = (m >= 0) and not on_vec
                            sp = psA.tile([128, 1024], f32, tag="A")
                            for ab in range(2):
                                nc.tensor.matmul(
                                    sp[:, ab * 512 + lo:(ab + 1) * 512],
                                    lhsT=kA[ab * 64:(ab + 1) * 64,
                                            i * 128:(i + 1) * 128],
                                    rhs=qA[ab * 64:(ab + 1) * 64,
                                           j * 512 + lo:(j + 1) * 512],
                                    start=True, stop=not mask_mm)
                            if mask_mm:
                                # scalar-exp diag blocks: one masking matmul
                                # covers both heads (the out AP picks the
                                # two 128-col diagonal sub-blocks, stride
                                # 512 apart); ACT exp maps -45 to 0.
                                dst2 = sp[:].rearrange(
                                    "p (a b) -> p a b", a=2)[:, :, lo:lo + 128]
                                nc.tensor.matmul(
                                    dst2, lhsT=nid, rhs=mk2,
                                    start=False, stop=True)
                            P = pP.tile([128, 1024], f16, tag="P")
                            # full-width exp, alternating engines per block
                            # (amortizes the ~340ns per-op engine overhead;
                            # with the depth-2 PV pipeline each engine gets
                            # ~2 block-times of latency slack). Vector-exp
                            # diag blocks fuse the causal mask into the
                            # Schraudolph op via the smk bias tile (the
                            # diag windows sit at fixed offsets 0 and 512
                            # of the [lo:1024] view), so they need no PE
                            # mask matmul at all.
                            if not on_vec:
                                nc.scalar.activation(
                                    P[:, lo:1024], sp[:, lo:1024], Exp)
                            elif m >= 0:
                                nc.vector.scalar_tensor_tensor(
                                    out=P[:, lo:1024].bitcast(i16),
                                    in0=sp[:, lo:1024], scalar=SCH_A,
                                    in1=smk[:, 0:1024 - lo],
                                    op0=Alu.mult, op1=Alu.add)
                            else:
                                nc.vector.tensor_scalar(
                                    out=P[:, lo:1024].bitcast(i16),
                                    in0=sp[:, lo:1024],
                                    scalar1=SCH_A, scalar2=SCH_B,
                                    op0=Alu.mult, op1=Alu.add)
                            Ps[i], ms[i] = P, max(m, 0)

                        def emit_PV(i):
                            m = ms[i]
                            lo = 128 * m
                            P = Ps[i]
                            first, last = (i == 0), (i == nblk - 1)
                            nc.tensor.matmul(
                                pvA[:, lo:512], lhsT=vo(i, 2 * p),
                                rhs=P[:, lo:512], start=first, stop=last)
                            nc.tensor.matmul(
                                pvB[:, lo:512], lhsT=vo(i, 2 * p + 1),
                                rhs=P[:, 512 + lo:1024], start=first, stop=last)

                        def norm_cols(cs, ce, tags):
                            # normalize columns [cs:ce) of this chunk: stage
                            # denominators fp16 (scalar+vector), half-swap
                            # via the PE permutation matmul, one reciprocal
                            # for both heads, multiply into yt.
                            w = ce - cs
                            s1 = pst.tile([128, w], f16, tag=tags[0],
                                          name="s1n")
                            nc.scalar.copy(s1[64:128, :], pvA[64:128, cs:ce])
                            nc.vector.tensor_copy(s1[0:64, :],
                                                  pvB[0:64, cs:ce])
                            s2p = psB.tile([128, 512], f32, tag="B",
                                           name="s2p")
                            nc.tensor.matmul(s2p[:, 0:w], lhsT=perm,
                                             rhs=s1[:], start=True, stop=True)
                            s3 = pst.tile([128, w], f32, tag=tags[1],
                                          name="s3n")
                            nc.vector.reciprocal_approx_fast(s3[:],
                                                             s2p[:, 0:w])
                            c0 = j * 512 + cs
                            nc.vector.tensor_mul(
                                yt[p][0:64, c0:c0 + w],
                                pvA[0:64, cs:ce], s3[0:64, :])
                            nc.vector.tensor_mul(
                                yt[p][64:128, c0:c0 + w],
                                pvB[64:128, cs:ce], s3[64:128, :])

                        last = (p == 2 and j == 3)

                        def endgame(mi):
                            # the last chunk's PV(12+mi) is the final writer
                            # of accumulator columns [mi*128:(mi+1)*128), so
                            # that slice can normalize and its proj tile can
                            # run while the remaining PVs are still going.
                            norm_cols(mi * 128, mi * 128 + 128,
                                      ("sl1", "sl3"))
                            emit_proj_t(12 + mi, big=True)

                        # software-pipeline, depth 2: PV(i) emitted after
                        # S(i+2), so exp(i) has ~2 block-times before the
                        # PE needs P(i), and PV(0) (which WAR-waits the
                        # previous chunk's normalize via psV) sits deep
                        # enough not to head-of-line block the PE.
                        emit_S(0)
                        emit_S(1)
                        pump(1)
                        for i in range(2, nblk):
                            emit_S(i)
                            emit_PV(i - 2)
                            pump(i)
                        emit_PV(nblk - 2)
                        if last:
                            while emitted[0] < nf:
                                fillers[emitted[0]]()
                                emitted[0] += 1
                            endgame(0)
                            emit_PV(nblk - 1)
                            endgame(1)
                            endgame(2)
                            endgame(3)
                        else:
                            emit_PV(nblk - 1)
                            pump(nblk)
                            while emitted[0] < nf:
                                fillers[emitted[0]]()
                                emitted[0] += 1
                            norm_cols(0, 512, ("st", "s3"))



    nc.compile()
    return nc


def _prep_inputs(x, W_qkv, b_qkv, W_proj):
    """Per-core input maps (numpy; all matmul operands float16).

    Everything is re-laid-out so each device DMA is a contiguous block:
      xe  [512, 3072]: row ch*128+p holds x^T cols [ch*512,(ch+1)*512) of
          all 6 contraction tiles (c-major).
      we  [128, 6912]: column groups [p0-q | p0-k | v | p12-q | p12-k],
          c-major inside each group.
      wpe [128, 2304]: wp row-block i at cols i*768.
      cm  [128, 1536]: [-45*I | tril(-1) | tril(-1) | half-swap perm |
                        Schraudolph bias+mask tile (1024)].
    """
    sc = 1.0 / np.sqrt(D)
    nid = (-45.0 * np.eye(128, dtype=np.float32))
    mkl = np.tril(np.ones((128, 128), dtype=np.float32), -1)
    i64 = np.eye(64, dtype=np.float32)
    z64 = np.zeros((64, 64), dtype=np.float32)
    prm = np.block([[z64, i64], [i64, z64]])
    # Schraudolph bias tile with the causal mask folded in: the [lo:1024]
    # exp window puts both diagonal sub-blocks at fixed view offsets 0 and
    # 512; masked entries get SCH_B - 45*SCH_A (saturates int16 -> -0.0).
    dmask = np.full((128, 128), SCH_B, dtype=np.float32)
    dmask[mkl.astype(bool)] = SCH_B - 45.0 * SCH_A
    flat = np.full((128, 384), SCH_B, dtype=np.float32)
    smk = np.concatenate([dmask, flat, dmask, flat], axis=1)
    cm = np.concatenate([nid, mkl, mkl, prm, smk], axis=1).astype(np.float16)
    in_maps = []
    for c in range(N_CORES):
        b, hh = c // 2, c % 2
        h0 = hh * 384                      # column offset of this half's heads
        wq = W_qkv[:, h0:h0 + 384] * sc
        wk = W_qkv[:, 768 + h0:768 + h0 + 384]
        wv = W_qkv[:, 1536 + h0:1536 + h0 + 384]
        wa = np.concatenate([wq, wk, wv], axis=1)        # [768, 1152]
        wr = wa.reshape(CT, 128, 1152)
        we = np.concatenate([
            wr[:, :, 0:128].transpose(1, 0, 2).reshape(128, 768),
            wr[:, :, 384:512].transpose(1, 0, 2).reshape(128, 768),
            wr[:, :, 768:1152].transpose(1, 0, 2).reshape(128, 2304),
            wr[:, :, 128:384].transpose(1, 0, 2).reshape(128, 1536),
            wr[:, :, 512:768].transpose(1, 0, 2).reshape(128, 1536),
        ], axis=1)
        xe = (np.ascontiguousarray(x[b].T).reshape(CT, 128, 4, 512)
              .transpose(2, 1, 0, 3).reshape(512, 3072))
        wpe = (W_proj[h0:h0 + 384, :].reshape(NP, 128, C)
               .transpose(1, 0, 2).reshape(128, NP * C))
        m = {
            "xe": np.ascontiguousarray(xe, dtype=np.float16),
            "we": np.ascontiguousarray(we, dtype=np.float16),
            "wpe": np.ascontiguousarray(wpe, dtype=np.float16),
            "cm": cm,
        }
        if np.any(b_qkv):
            bq = b_qkv[h0:h0 + 384] * sc
            bk = b_qkv[768 + h0:768 + h0 + 384]
            bv = b_qkv[1536 + h0:1536 + h0 + 384]
            m["ox"] = np.ones((1, T), dtype=np.float16)
            m["wb"] = np.concatenate([bq, bk, bv]).reshape(1, 1152).astype(
                np.float16)
        in_maps.append(m)
    return in_maps


def _run(inputs, trace=False, tmpdir=None):
    from concourse.bass_utils import run_bass_kernel_spmd

    x = np.asarray(inputs["x"], dtype=np.float32)
    W_qkv = np.asarray(inputs["W_qkv"], dtype=np.float32)
    b_qkv = np.asarray(inputs["b_qkv"], dtype=np.float32)
    W_proj = np.asarray(inputs["W_proj"], dtype=np.float32)
    b_proj = np.asarray(inputs["b_proj"], dtype=np.float32)

    has_bias = bool(np.any(b_qkv))
    key = ("k", has_bias)
    if key not in _cache:
        _cache[key] = _build(has_bias)
    nc = _cache[key]

    in_maps = _prep_inputs(x, W_qkv, b_qkv, W_proj)
    res = run_bass_kernel_spmd(nc, in_maps, list(range(N_CORES)),
                               trace=trace, tmpdir=tmpdir)
    out = np.empty((B, T, C), dtype=np.float32)
    for b in range(B):
        out[b] = (res.results[2 * b]["out"].astype(np.float32)
                  + res.results[2 * b + 1]["out"].astype(np.float32))
    out += b_proj
    return out, res


def kernel(**inputs):
    out, _ = _run(inputs)
    return out
